# revision 83
# baseline (speedup 1.0000x reference)
"""Dilated self-attention Trainium2 kernel (8-core SPMD).

Problem (hardcoded): x [4, 8192, 256], Wq/Wk/Wv [256, 256] f32.
WS=[2048,4096,8192], RS=[1,2,4], HEAD_IDX=0 -> every config has segment
length 2048 after dilation; 28 segments total.

Sharding: core = (b, h) with b in 0..3, h in 0..1. Core (b,h) owns output
tokens [4096h, 4096h+4096) of batch b and computes the 4 attention
segments that contribute to them:
  seg0 = config1 seg 2h   (tokens 4096h+[0,2048))
  seg1 = config1 seg 2h+1 (tokens 4096h+[2048,4096))
  seg2 = config2 seg h    (tokens 4096h+(0,2,4,...) -- 2048 even rows)
  seg3 = config3 seg 0    (tokens 0::4 over the whole batch, computed
                           fully on both cores of the pair; each core
                           uses only its half of the rows, selected with
                           a runtime register offset so the SPMD program
                           is identical across cores)
Per-token combine (sum of unnormalized outputs / sum of denominators)
is then fully core-local; no collectives.

Layout: host passes x pre-transposed per segment (xsT [4,256,2048]) and
transposed weights; the kernel computes GT = Wk @ Wq^T once, then per
segment HT = G @ X^T and V = X @ Wv, and scores_T[k,q] blocks with flash
accumulation. The causal diagonal supertile is refined at 256-column
granularity (skipping the two fully-masked k-tiles of the left half),
which saves ~11% of PE cycles vs 512-wide-everywhere. Output is produced
transposed ([256, 4096]) and un-transposed on the host.
"""

import os
import numpy as np

import concourse.bass as bass
import concourse.mybir as mybir
import concourse.tile as tile
from concourse import bacc
from concourse.bass_utils import run_bass_kernel_spmd
from concourse.masks import make_identity

F32 = mybir.dt.float32
F32R = mybir.dt.float32r
I32 = mybir.dt.int32
AF = mybir.ActivationFunctionType

B, N, C, D = 4, 8192, 256, 256
SEG = 2048          # segment length (rows) for every config
P = 128             # partitions
NT = SEG // P       # 16 k-tiles per segment
QST = 512           # q supertile width
NJ = SEG // QST     # 4 q supertiles per segment
HALF = N // 2       # 4096 tokens owned per core
NSEG = 4            # segments per core
MASK_VAL = -20000.0
SCALE = 1.0 / 16.0  # 1/sqrt(D)

BF16 = mybir.dt.bfloat16
USE_BF16 = os.environ.get("USE_BF16", "1") == "1"
MMDT = BF16 if USE_BF16 else F32R
EDT = MMDT
USE_SWITCH = os.environ.get("USE_SWITCH", "1") == "1"
ODT = BF16 if USE_BF16 else F32


def _emit(tc, static_c3off=None, static_reps=None):
    nc = tc.nc

    xsT_d = nc.dram_tensor("xsT", [NSEG, C, SEG], MMDT, kind="ExternalInput").ap()
    wqT_d = nc.dram_tensor("wqT", [D, C], MMDT, kind="ExternalInput").ap()
    wkT_d = nc.dram_tensor("wkT", [D, C], MMDT, kind="ExternalInput").ap()
    wv_d = nc.dram_tensor("wv", [C, D], MMDT, kind="ExternalInput").ap()
    c3off_d = nc.dram_tensor("c3off", [1, 1], I32, kind="ExternalInput").ap()
    harg_d = nc.dram_tensor("harg", [1, 1], I32, kind="ExternalInput").ap()
    reps_d = nc.dram_tensor("reps", [1, 1], I32, kind="ExternalInput").ap()
    outT_d = nc.dram_tensor("outT", [C, HALF], ODT, kind="ExternalOutput").ap()

    import contextlib
    ctx = contextlib.ExitStack()
    with ctx:
        consts = ctx.enter_context(tc.tile_pool(name="consts", bufs=1))
        big = ctx.enter_context(tc.tile_pool(name="big", bufs=1))
        xt_pool = ctx.enter_context(tc.tile_pool(name="xt", bufs=2))
        e_pool = ctx.enter_context(tc.tile_pool(name="e", bufs=int(__import__("os").environ.get("E_BUFS", "3")) if USE_BF16 else 2))
        pr_pool = ctx.enter_context(tc.tile_pool(name="pr", bufs=2))
        ps_sc = ctx.enter_context(tc.tile_pool(name="ps_sc", bufs=int(__import__("os").environ.get("PSC_BUFS", "3")), space="PSUM"))
        ps_po = ctx.enter_context(tc.tile_pool(name="ps_po", bufs=2, space="PSUM"))
        ps_pd = ctx.enter_context(tc.tile_pool(name="ps_pd", bufs=int(__import__("os").environ.get("PD_BUFS", "1")), space="PSUM"))

        # ---- DMA plan: SP queue carries the loop gate (reps) + xT loads;
        # Act queue carries weights (one strided DMA per tensor) + scalars.
        reps_sb = consts.tile([1, 1], I32, tag="reps")
        nc.sync.dma_start(reps_sb, reps_d)
        wq_all = consts.tile([P, 2, C], MMDT, tag="wq_all")
        nc.sync.dma_start(wq_all, wqT_d.rearrange("(f p) c -> p f c", p=P))
        wk_all = consts.tile([P, 2, C], MMDT, tag="wk_all")
        nc.scalar.dma_start(wk_all, wkT_d.rearrange("(f p) c -> p f c", p=P))
        # prefetch the first segment (s=2) outside the reps loop; the body
        # reloads this tile at its end so reps>1 iterations stay correct.
        xt_pre = consts.tile([P, 2, SEG], MMDT, tag="xt_pre")
        xsT_src2 = xsT_d[2].rearrange("(c p) q -> p c q", p=P)
        for half in range(2):
            cols = slice(SEG // 2 * half, SEG // 2 * (half + 1))
            nc.sync.dma_start(xt_pre[:, :, cols], xsT_src2[:, :, cols])
        wv_all = consts.tile([P, 2, D], MMDT, tag="wv_all")
        nc.scalar.dma_start(wv_all, wv_d.rearrange("(f p) c -> p f c", p=P))
        c3off_sb = consts.tile([1, 1], I32, tag="c3off")
        nc.scalar.dma_start(c3off_sb, c3off_d)
        harg_sb = consts.tile([1, 1], I32, tag="harg")
        nc.scalar.dma_start(harg_sb, harg_d)
        wqT_sb = [wq_all[:, i, :] for i in range(2)]
        wkT_sb = [wk_all[:, i, :] for i in range(2)]
        wv_sb = [wv_all[:, i, :] for i in range(2)]

        ident_f = consts.tile([P, P], F32, tag="mscratch", name="ident_f")
        make_identity(nc, ident_f)
        ident = consts.tile([P, P], EDT, tag="ident")
        nc.vector.tensor_copy(ident, ident_f)

        ones_f = consts.tile([P, P], F32, tag="ones_f")
        nc.vector.memset(ones_f, 1.0)
        ones_col = consts.tile([P, P], EDT, tag="ones_col")
        nc.vector.tensor_copy(ones_col, ones_f)
        ones_row = consts.tile([1, P], F32, tag="ones_row")
        nc.vector.memset(ones_row, 1.0)

        # diagonal additive mask [128, 128]: M[kr, qc] = 0 if qc >= kr else MASK_VAL
        mf = consts.tile([P, P], F32, tag="mscratch2", name="mask_f")
        nc.gpsimd.memset(mf, 0.0)
        nc.gpsimd.affine_select(
            out=mf, in_=mf, compare_op=mybir.AluOpType.is_ge,
            fill=MASK_VAL, base=0, channel_multiplier=-1,
            pattern=[[1, P]],
        )
        mask = consts.tile([P, P], EDT, tag="mask", name="mask")
        nc.vector.tensor_copy(mask, mf)

        # GT = Wk @ Wq^T  [256, 256]  (= (Wq Wk^T)^T)
        GT_sb = [consts.tile([P, C], MMDT, tag=f"GT{i}", name=f"GT{i}") for i in range(2)]
        for a in range(2):  # output row chunk
            ps = ps_sc.tile([P, QST], F32, tag="psc", name="gtps")[:, 0:C]
            for dch in range(2):
                nc.tensor.matmul(
                    ps, wkT_sb[dch][:, P * a:P * (a + 1)], wqT_sb[dch],
                    start=(dch == 0), stop=(dch == 1))
            nc.vector.tensor_copy(GT_sb[a], ps)

        # loaded after the GT emission so the TensorLoad doesn't block GT's
        # matmuls in the in-order PE instruction stream
        if static_reps is None:
            reps_v = nc.values_load(reps_sb, min_val=1, max_val=10000, skip_runtime_bounds_check=True)
        else:
            reps_v = static_reps

        rv = {}  # runtime values loaded at body start (c3v, h_v)

        # ---- persistent per-iteration state ----
        # oT[s][c]: unnormalized attention output, transposed: [128, 2048] per
        # (segment s, feature chunk c). den[s]: [1, 2048].
        oT = big.tile([P, NSEG, 2, SEG], F32, tag="oT")
        den = big.tile([1, NSEG, SEG], F32, tag="den")

        def _proj(s, xT):
            """HT = G @ X^T [256,2048] and V = X @ Wv [2048,256] for segment."""
            HT = [xt_pool.tile([P, SEG], MMDT, tag=f"HT{c}", name=f"HT{c}", bufs=1)
                  for c in range(2)]
            for fo in range(2):
                for r in range(NJ):
                    ps = ps_sc.tile([P, QST], F32, tag="psc", name=f"htps{fo}{r}")
                    for fi in range(2):
                        nc.tensor.matmul(
                            ps,
                            GT_sb[fi][:, P * fo:P * (fo + 1)],
                            xT[fi][:, QST * r:QST * (r + 1)],
                            start=(fi == 0), stop=(fi == 1))
                    dst = HT[fo][:, QST * r:QST * (r + 1)]
                    if (fo * NJ + r) % 2 == 0:
                        nc.scalar.copy(dst, ps)
                    else:
                        nc.vector.tensor_copy(dst, ps)

            V = xt_pool.tile([P, NT, D], EDT, tag="V", bufs=1)
            for kq in range(NT // 2):
                ps = ps_sc.tile([P, QST], F32, tag="psc", name=f"vps{kq}")
                for idx in range(2):
                    kt = 2 * kq + idx
                    for fi in range(2):
                        nc.tensor.matmul(
                            ps[:, D * idx:D * (idx + 1)],
                            xT[fi][:, P * kt:P * (kt + 1)],
                            wv_sb[fi],
                            start=(fi == 0), stop=(fi == 1))
                dstv = V[:, 2 * kq:2 * (kq + 1), :].rearrange("p a b -> p (a b)")
                if kq % 2 == 0:
                    nc.vector.tensor_copy(dstv, ps)
                else:
                    nc.scalar.copy(dstv, ps)
            return HT, V

        def _attn_block(s, j, xT, HT, V):
            """Causal attention for q supertile j (512 cols) of segment s.

            Off-diagonal k-tiles (kt < 4j) run 512 wide; the diagonal
            supertile is refined at 256 columns: left half A needs only
            kt 4j..4j+1 (masked t0,t1), right half B needs kt 4j..4j+1
            unmasked plus 4j+2..4j+3 masked t0,t1.
            """
            po = [ps_po.tile([P, QST], F32, tag=f"po{c}", name=f"po{c}_{j}")
                  for c in range(2)]
            pd = ps_pd.tile([P, QST], F32, tag="pd", name=f"pd{j}")
            qlo = QST * j

            def eV(e_ap, cols, kts, first, last):
                # accumulate po[:, cols] += V[kt]^T e ; pd[:, cols] += 1^T e
                ncols = cols.stop - cols.start
                for i, kt in enumerate(kts):
                    ei = e_ap[:, i * ncols:(i + 1) * ncols]
                    st = first and i == 0
                    sp = last and i == len(kts) - 1
                    for c in range(2):
                        nc.tensor.matmul(
                            po[c][:, cols], V[:, kt, P * c:P * (c + 1)], ei,
                            start=st, stop=sp, skip_group_check=True)
                    nc.tensor.matmul(
                        pd[:, cols], ones_col, ei,
                        start=st, stop=sp, skip_group_check=True)

            # --- off-diagonal: kt in [0, 4j), 512-wide, one k-tile each ---
            for kt in range(4 * j):
                psc = ps_sc.tile([P, QST], F32, tag="psc", name=f"sc{j}_{kt}")
                nc.tensor.matmul(
                    psc, HT[0][:, P * kt:P * (kt + 1)],
                    xT[0][:, qlo:qlo + QST], start=True, stop=False)
                nc.tensor.matmul(
                    psc, HT[1][:, P * kt:P * (kt + 1)],
                    xT[1][:, qlo:qlo + QST], start=False, stop=True)
                e = e_pool.tile([P, QST], EDT, tag="e")
                nc.scalar.activation(e, psc, AF.Exp, scale=SCALE)
                eV(e, slice(0, QST), [kt], first=(kt == 0), last=False)

            # --- diagonal supertile, 128-refined ---
            # q sub-tile t (128 cols) needs k-tiles 4j+i for i <= t; the
            # i == t block is masked with the [128,128] causal mask. Only
            # 10 of 16 sub-blocks are computed.
            for t in range(4):
                pscD = ps_sc.tile([P, 4, P], F32, tag="psc", name=f"scD{j}_{t}")
                qq = qlo + P * t
                for i in range(t + 1):
                    kt = 4 * j + i
                    diag = i == t
                    for fi in range(2):
                        nc.tensor.matmul(
                            pscD[:, i, :], HT[fi][:, P * kt:P * (kt + 1)],
                            xT[fi][:, qq:qq + P],
                            start=(fi == 0), stop=(fi == 1 and not diag))
                    if diag:
                        nc.tensor.matmul(
                            pscD[:, i, :], ident, mask, start=False, stop=True)
                eD = e_pool.tile([P, 4, P], EDT, tag="eD")
                nc.scalar.activation(eD[:, 0:t + 1, :], pscD[:, 0:t + 1, :],
                                     AF.Exp, scale=SCALE)
                for i in range(t + 1):
                    kt = 4 * j + i
                    st = (j == 0 and i == 0)
                    sp = (i == t)
                    for c in range(2):
                        nc.tensor.matmul(
                            po[c][:, P * t:P * t + P],
                            V[:, kt, P * c:P * (c + 1)], eD[:, i, :],
                            start=st, stop=sp, skip_group_check=True)
                    nc.tensor.matmul(
                        pd[:, P * t:P * t + P], ones_col, eD[:, i, :],
                        start=st, stop=sp, skip_group_check=True)
            # evacuate (oT on Act, den on DVE, to keep DVE off the po-release path)
            for c in range(2):
                nc.scalar.copy(oT[:, s, c, qlo:qlo + QST], po[c])
            nc.vector.tensor_copy(den[:, s, qlo:qlo + QST], pd[0:1, :])

        def _combine_pre(s, j):
            """Precompute the c2+c3 contributions for block j of seg s into
            scratch tiles (runs during the block's attention)."""
            lo = QST * j
            g = SEG * s + lo
            sc_o = pr_pool.tile([P, 2, QST], F32, tag="sc_o")
            sc_d = pr_pool.tile([1, QST], F32, tag="sc_d")
            nc.gpsimd.memset(sc_o, 0.0)
            nc.vector.memset(sc_d, 0.0)
            d2 = sc_d.rearrange("p (q two) -> p q two", two=2)[:, :, 0:1]
            nc.vector.tensor_add(
                d2, d2, den[:, 2, g // 2:g // 2 + QST // 2].rearrange(
                    "p (q one) -> p q one", one=1))
            d4 = sc_d.rearrange("p (q four) -> p q four", four=4)[:, :, 0:1]
            nc.vector.tensor_add(
                d4, d4, den[:, 3, bass.ds(rv["c3v"] + g // 4, QST // 4)].rearrange(
                    "p (q one) -> p q one", one=1))
            for c in range(2):
                o2 = sc_o[:, c, :].rearrange("p (q two) -> p q two", two=2)[:, :, 0:1]
                nc.gpsimd.tensor_add(
                    o2, o2, oT[:, 2, c, g // 2:g // 2 + QST // 2].rearrange(
                        "p (q one) -> p q one", one=1))
                o4 = sc_o[:, c, :].rearrange("p (q four) -> p q four", four=4)[:, :, 0:1]
                nc.gpsimd.tensor_add(
                    o4, o4, oT[:, 3, c, bass.ds(rv["c3v"] + g // 4, QST // 4)].rearrange(
                        "p (q one) -> p q one", one=1))
            return sc_o, sc_d

        def _combine_block(s, j, tail=False, pre=None):
            # fold config2 (even tokens) and config3 (every 4th) into block j
            # of seg s, divide by the summed denominator, store transposed.
            # The denominator chain (DVE adds -> Pool broadcast -> DVE recip)
            # runs concurrently with the Pool oT adds; muls+DMA pipelined per c.
            lo = QST * j
            g = SEG * s + lo            # token offset inside the half
            dstd = den[:, s, lo:lo + QST]
            if pre is not None:
                sc_o, sc_d = pre
                nc.vector.tensor_add(dstd, dstd, sc_d)
            else:
                dd2 = dstd.rearrange("p (q two) -> p q two", two=2)[:, :, 0:1]
                nc.vector.tensor_add(
                    dd2, dd2,
                    den[:, 2, g // 2:g // 2 + QST // 2].rearrange(
                        "p (q one) -> p q one", one=1))
                dd4 = dstd.rearrange("p (q four) -> p q four", four=4)[:, :, 0:1]
                nc.vector.tensor_add(
                    dd4, dd4,
                    den[:, 3, bass.ds(rv["c3v"] + g // 4, QST // 4)].rearrange(
                        "p (q one) -> p q one", one=1))
            pr = pr_pool.tile([P, QST], F32, tag="pr")
            if tail:
                # PE is idle at the very end: broadcast via matmul with a
                # ones column instead of the slower Pool broadcast.
                prp = ps_pd.tile([P, QST], F32, tag="pd", name="prp")
                nc.tensor.matmul(prp, ones_row, dstd, start=True, stop=True)
                nc.vector.reciprocal(pr, prp)
            else:
                nc.gpsimd.partition_broadcast(pr, dstd)
                nc.vector.reciprocal(pr, pr)
            obf = pr_pool.tile([P, 2, QST], ODT, tag="obf")
            for c in range(2):
                dst = oT[:, s, c, lo:lo + QST]
                if pre is not None:
                    nc.gpsimd.tensor_add(dst, dst, sc_o[:, c, :])
                else:
                    d2 = dst.rearrange("p (q two) -> p q two", two=2)[:, :, 0:1]
                    nc.gpsimd.tensor_add(
                        d2, d2,
                        oT[:, 2, c, g // 2:g // 2 + QST // 2].rearrange(
                            "p (q one) -> p q one", one=1))
                    d4 = dst.rearrange("p (q four) -> p q four", four=4)[:, :, 0:1]
                    nc.gpsimd.tensor_add(
                        d4, d4,
                        oT[:, 3, c, bass.ds(rv["c3v"] + g // 4, QST // 4)].rearrange(
                            "p (q one) -> p q one", one=1))
                nc.vector.tensor_mul(obf[:, c, :], dst, pr)
                if tail:
                    # split the final store so chunk 0 streams during chunk
                    # 1's multiply
                    nc.sync.dma_start(
                        outT_d[P * c:P * (c + 1), g:g + QST], obf[:, c, :])
            if not tail:
                # one strided DMA for both feature chunks; SP queue is idle
                # here (xT loads issued during segs 2/3) so it can't block exps
                nc.sync.dma_start(
                    outT_d[:, g:g + QST].rearrange("(c p) q -> p c q", p=P),
                    obf)

        def body(_iv):
            pending = [None]
            if static_c3off is None:
                rv["c3v"] = nc.values_load(c3off_sb, min_val=0, max_val=SEG // 2, skip_runtime_bounds_check=True)
                rv["h_v"] = nc.values_load(harg_sb, min_val=0, max_val=1, skip_runtime_bounds_check=True)
            else:
                rv["c3v"] = static_c3off
                rv["h_v"] = 1 if static_c3off else 0
            for s in (2, 3, 0, 1):
                if s == 2:
                    xt_all = xt_pre
                else:
                    xt_all = xt_pool.tile([P, 2, SEG], MMDT, tag="xt",
                                          name=f"xt{s}",
                                          bufs=2)
                    src = xsT_d[s].rearrange("(c p) q -> p c q", p=P)
                    for half in range(2):
                        cols = slice(SEG // 2 * half, SEG // 2 * (half + 1))
                        nc.sync.dma_start(xt_all[:, :, cols], src[:, :, cols])
                xT = [xt_all[:, c, :] for c in range(2)]
                if s == 1:
                    # reload the prefetch tile for the next rep while seg 1
                    # computes (its last reader, seg 2, finished long ago)
                    for half in range(2):
                        cols = slice(SEG // 2 * half, SEG // 2 * (half + 1))
                        nc.sync.dma_start(xt_pre[:, :, cols], xsT_src2[:, :, cols])
                HT, V = _proj(s, xT)
                if pending[0] is not None:
                    # deferred last-block combine of the previous segment:
                    # emitted after this segment's projection so its DVE work
                    # doesn't block the HT/V psum evacuations.
                    pending[0]()
                    pending[0] = None
                if s == 3 and static_c3off is None and USE_SWITCH:
                    # config3: core h only needs output rows for its own token
                    # half -> q supertiles {2h, 2h+1}. Dispatch per core.
                    for arm in tc.Switch(rv["h_v"], 2):
                        for j in (2 * arm, 2 * arm + 1):
                            _attn_block(s, j, xT, HT, V)
                elif s == 3 and static_c3off is not None:
                    for j in (2 * rv["h_v"], 2 * rv["h_v"] + 1):
                        _attn_block(s, j, xT, HT, V)
                else:
                    for j in range(NJ):
                        is_tail = s == 1 and j == NJ - 1
                        pre = _combine_pre(s, j) if is_tail else None
                        _attn_block(s, j, xT, HT, V)
                        if s in (0, 1):
                            if j == NJ - 1 and s == 0:
                                pending[0] = (lambda ss=s, jj=j:
                                              _combine_block(ss, jj))
                            else:
                                _combine_block(s, j, tail=is_tail, pre=pre)

        if static_reps == 1:
            body(0)
        else:
            with tc.For_i(0, reps_v) as iv:
                body(iv)


_NC_CACHE = None


def _get_nc():
    global _NC_CACHE
    if _NC_CACHE is None:
        nc = bacc.Bacc("TRN2", target_bir_lowering=False, debug=False,
                       num_devices=8)
        with tile.TileContext(nc) as tc:
            _emit(tc)
        nc.compile()
        _NC_CACHE = nc
    return _NC_CACHE


def _make_in_maps(x, Wq, Wk, Wv, reps=1):
    if USE_BF16:
        from ml_dtypes import bfloat16 as indt
    else:
        indt = np.float32
    wqT = np.ascontiguousarray(Wq.T).astype(indt)
    wkT = np.ascontiguousarray(Wk.T).astype(indt)
    wv = np.ascontiguousarray(Wv).astype(indt)
    in_maps = []
    for core in range(8):
        b, h = core // 2, core % 2
        xb = x[b]                                  # [8192, 256]
        xa = xb[HALF * h:HALF * (h + 1)]           # [4096, 256]
        segs = [
            xa[0:SEG],                             # config1 seg 2h
            xa[SEG:2 * SEG],                       # config1 seg 2h+1
            xa[0::2],                              # config2 seg h
            xb[0::4],                              # config3 (full)
        ]
        xsT = np.ascontiguousarray(
            np.stack([s.T for s in segs], axis=0)).astype(indt)
        in_maps.append({
            "xsT": xsT,
            "wqT": wqT,
            "wkT": wkT,
            "wv": wv,
            "c3off": np.array([[(SEG // 2) * h]], dtype=np.int32),
            "harg": np.array([[h]], dtype=np.int32),
            "reps": np.array([[reps]], dtype=np.int32),
        })
    return in_maps


def run_cores(x, Wq, Wk, Wv, reps=1):
    nc = _get_nc()
    in_maps = _make_in_maps(x, Wq, Wk, Wv, reps=reps)
    res = run_bass_kernel_spmd(nc, in_maps, core_ids=list(range(8)))
    return res


def kernel(x, Wq, Wk, Wv):
    x = np.asarray(x, dtype=np.float32)
    res = run_cores(x, np.asarray(Wq, np.float32), np.asarray(Wk, np.float32),
                    np.asarray(Wv, np.float32))
    out = np.empty((B, N, D), dtype=np.float32)
    for core in range(8):
        b, h = core // 2, core % 2
        out[b, HALF * h:HALF * (h + 1), :] = \
            res.results[core]["outT"].astype(np.float32).T
    return out
